# revision 8
# baseline (speedup 1.0000x reference)
"""Trainium2 Bass kernel for the LGP-instruction module (read -> op bank -> write).

Data-parallel over batch: core b computes x[b] (2048, 4096).
All HBM traffic in bf16 (x, params, out) -- the problem is memory-bound, so
halving bytes halves the roofline; rel-err budget (2e-2) easily covers bf16.

Device pipeline per core, per T-chunk of 512 (software-pipelined so PE never
idles long enough for HAM to re-throttle):
  phase 1: values[C, 512] = sum_vt rw[vt].T @ x_tile[vt]    (PSUM f32, bf16 MMs)
  phase 2: h_k = W_k.T @ vals (PSUM) -> ACT f_k(h + b_k) -> DVE bf16 accumulate
           (identity and neg ops are pre-merged on host: 7 effective ops)
  phase 3: out[128T, V] = acc.T @ wwT  (PSUM f32) -> 1024-wide drains -> stores

Host prep: read_w softmax, write_w*out_scale transpose, mixture weights folded
into op weights/biases, x pre-tiled to the exact SBUF layout in bf16 so every
load is one contiguous descriptor set. Output returned bf16, upcast on host.
"""
import sys
import numpy as np

if '/opt/trn_rl_repo' not in sys.path:
    sys.path.insert(0, '/opt/trn_rl_repo')

B, T, V, C, NOPS = 8, 2048, 4096, 128, 8
NCORES = 8
NV = V // 128     # 32 v-tiles
NTC = T // 512    # 4 T-chunks
NBLK = 2          # x load blocks per T-chunk (2MB each)
VB = NV // NBLK   # 16 v-tiles per block
NK = 7            # effective ops after identity+neg merge
ACT_SET = {1, 3, 6, 8, 10, 13, 15}   # 7/16 psum drains to ACT

_CACHE = {}
LAST_RESULT = None


def _build(post):
    from concourse import bass, bacc, tile, mybir
    f32, bf16 = mybir.dt.float32, mybir.dt.bfloat16
    fp8 = mybir.dt.float8e4
    AF = mybir.ActivationFunctionType
    ts = bass.ts
    FUNCS = [AF.Identity, AF.Relu, AF.Gelu, AF.Square,
             AF.Abs, AF.Tanh, AF.Sigmoid]

    nc = bacc.Bacc("TRN2", target_bir_lowering=False, debug=False,
                   num_devices=NCORES)
    xh = nc.dram_tensor("xh", [128, NTC * NBLK * VB * 512], fp8,
                        kind="ExternalInput")
    rw = nc.dram_tensor("rw", [128, NV * C], bf16, kind="ExternalInput")
    wwT = nc.dram_tensor("wwT", [C, V], bf16, kind="ExternalInput")
    opw = nc.dram_tensor("opw", [C, NK * C], bf16, kind="ExternalInput")
    opb = nc.dram_tensor("opb", [C, NK], f32, kind="ExternalInput")
    fp8o = mybir.dt.float8e5
    out = nc.dram_tensor("out", [T, V], fp8o, kind="ExternalOutput")
    asum = nc.dram_tensor("asum", [C, NTC], f32, kind="ExternalOutput")
    out_r = out.ap().rearrange("(r p) v -> p r v", p=128)

    with tile.TileContext(nc) as tc:
        with tc.tile_pool(name="const", bufs=1) as constp, \
             tc.tile_pool(name="xt", bufs=4) as xtp, \
             tc.tile_pool(name="vals_ps", bufs=2, space="PSUM") as vpsp, \
             tc.tile_pool(name="vals_sb", bufs=2) as vsbp, \
             tc.tile_pool(name="h_ps", bufs=2, space="PSUM") as hpsp, \
             tc.tile_pool(name="t_sb", bufs=3) as tp, \
             tc.tile_pool(name="acc", bufs=2) as accp, \
             tc.tile_pool(name="dacc", bufs=2) as daccp, \
             tc.tile_pool(name="mean", bufs=2) as meanp, \
             tc.tile_pool(name="out_ps", bufs=2, space="PSUM") as opsp, \
             tc.tile_pool(name="out_sb", bufs=4) as osbp:

            # rw first on the SP ring (needed by the very first matmul);
            # remaining consts go via the ACT HWDGE ring so x loads aren't
            # queued behind them.
            rw_t = constp.tile([128, NV * C], bf16)
            nc.sync.dma_start(rw_t[:, ts(0, NV * C // 4)],
                              rw.ap()[:, ts(0, NV * C // 4)])
            opb_t = constp.tile([C, NK], f32)
            nc.scalar.dma_start(opb_t[:], opb.ap())
            opw_t = constp.tile([C, NK * C], bf16)
            nc.scalar.dma_start(opw_t[:], opw.ap())
            wwT_t = constp.tile([C, V], bf16)
            nc.scalar.dma_start(wwT_t[:], wwT.ap())

            asum_t = constp.tile([C, NTC], f32)

            vals_sb = [None] * NTC

            def phase1(tcn):
                # read: accumulate over all V into one psum bank
                values = vpsp.tile([128, 512], f32)
                if tcn == 0:
                    # split the first blocks so the first MMs start earlier
                    blocks = [(0, 4), (4, 4), (8, 8), (16, 16)]
                else:
                    blocks = [(0, VB), (VB, VB)]
                for bi, (v0, nvb) in enumerate(blocks):
                    xt = xtp.tile([128, nvb * 512], fp8)
                    start_el = (tcn * NV + v0) * 512
                    assert start_el % (nvb * 512) == 0
                    nc.sync.dma_start(
                        xt[:], xh.ap()[:, ts(start_el // (nvb * 512),
                                             nvb * 512)])
                    if tcn == 0 and bi == 0:
                        # rw pieces 1-3 queue behind the first x piece
                        for q in range(1, 4):
                            nc.sync.dma_start(
                                rw_t[:, ts(q, NV * C // 4)],
                                rw.ap()[:, ts(q, NV * C // 4)])
                    for j in range(nvb):
                        vt = v0 + j
                        nc.tensor.matmul(values[:], rw_t[:, ts(vt, C)],
                                         xt[:, ts(j, 512)],
                                         start=(vt == 0), stop=(vt == NV - 1))
                vals = vsbp.tile([128, 512], bf16)
                nc.vector.tensor_copy(vals[:], values[:])
                vals_sb[tcn] = vals

            def opbank(tcn):
                vals = vals_sb[tcn]
                acc = accp.tile([128, 512], bf16)
                for k in range(NK):
                    h = hpsp.tile([128, 512], f32)
                    nc.tensor.matmul(h[:], opw_t[:, ts(k, C)], vals[:],
                                     start=True, stop=True)
                    if k == 0:
                        nc.scalar.activation(acc[:], h[:], FUNCS[0],
                                             bias=opb_t[:, 0:1])
                    else:
                        t = tp.tile([128, 512], bf16)
                        nc.scalar.activation(t[:], h[:], FUNCS[k],
                                             bias=opb_t[:, k:k + 1])
                        nc.vector.scalar_tensor_tensor(
                            acc[:], t[:], post[k], acc[:],
                            op0=mybir.AluOpType.mult, op1=mybir.AluOpType.add)
                # per-chunk mean over t: negsum -> negmean; ACT subtracts via
                # bias; host adds (-asum/512) @ wwT back.
                nc.vector.tensor_reduce(asum_t[:, tcn:tcn + 1], acc[:],
                                        axis=mybir.AxisListType.X,
                                        op=mybir.AluOpType.add, negate=True)
                negmean = meanp.tile([C, 1], f32)
                nc.vector.tensor_scalar_mul(negmean[:], asum_t[:, tcn:tcn + 1],
                                            1.0 / 512)
                dacc = daccp.tile([128, 512], bf16)
                nc.scalar.activation(dacc[:], acc[:], FUNCS[0],
                                     bias=negmean[:])
                return dacc

            def write(tcn, acc):
                for sub2 in range(2):
                    osb = osbp.tile([128, 2, V], fp8o)
                    for s in range(2):
                        sub = sub2 * 2 + s
                        for nn2 in range(4):
                            ops2 = opsp.tile([128, 1024], f32)
                            nc.tensor.matmul(ops2[:, 0:512],
                                             acc[:, ts(sub, 128)],
                                             wwT_t[:, ts(nn2 * 2, 512)],
                                             start=True, stop=True)
                            nc.tensor.matmul(ops2[:, 512:1024],
                                             acc[:, ts(sub, 128)],
                                             wwT_t[:, ts(nn2 * 2 + 1, 512)],
                                             start=True, stop=True)
                            if (sub * 4 + nn2) in ACT_SET:
                                nc.scalar.copy(osb[:, s, ts(nn2, 1024)],
                                               ops2[:])
                            else:
                                nc.vector.tensor_copy(osb[:, s, ts(nn2, 1024)],
                                                      ops2[:])
                    nc.gpsimd.dma_start(
                        out_r[:, ts(tcn * 2 + sub2, 2), :], osb[:])

            phase1(0)
            for c in range(NTC):
                acc = opbank(c)
                if c + 1 < NTC:
                    phase1(c + 1)
                write(c, acc)
            nc.gpsimd.dma_start(asum.ap(), asum_t[:])
    nc.compile()
    return nc


def _softmax(x, axis):
    x = np.asarray(x, np.float32)
    m = x.max(axis=axis, keepdims=True)
    e = np.exp(x - m)
    return e / e.sum(axis=axis, keepdims=True)


def kernel(x, basis, read_coeffs, write_coeffs, op_logits, op_weights,
           op_biases, out_scale):
    global LAST_RESULT
    import ml_dtypes
    from concourse.bass_utils import run_bass_kernel_spmd
    bf16 = ml_dtypes.bfloat16

    x = np.asarray(x, np.float32)
    basis = np.asarray(basis, np.float32)
    read_coeffs = np.asarray(read_coeffs, np.float32)
    write_coeffs = np.asarray(write_coeffs, np.float32)
    op_logits = np.asarray(op_logits, np.float32)
    op_weights = np.asarray(op_weights, np.float64)
    op_biases = np.asarray(op_biases, np.float64)
    out_scale = np.float32(out_scale)

    read_w = _softmax(basis @ read_coeffs.T, axis=0)               # (V, C)
    wwT = np.ascontiguousarray((basis @ write_coeffs.T).T) * out_scale  # (C, V)
    w = _softmax(op_logits, axis=0).astype(np.float64)

    # Fold mixture weights into op weights/biases where the nonlinearity
    # allows; merge the two linear ops (identity, neg) into one.
    #   orig i: 0 ident, 1 relu, 2 gelu, 3 square, 4 neg, 5 abs, 6 tanh, 7 sigm
    Wm = [w[0] * op_weights[0] - w[4] * op_weights[4],
          w[1] * op_weights[1],
          op_weights[2],
          np.sqrt(w[3]) * op_weights[3],
          w[5] * op_weights[5],
          op_weights[6],
          op_weights[7]]
    bm = [w[0] * op_biases[0] - w[4] * op_biases[4],
          w[1] * op_biases[1],
          op_biases[2],
          np.sqrt(w[3]) * op_biases[3],
          w[5] * op_biases[5],
          op_biases[6],
          op_biases[7]]
    post = [1.0, 1.0, float(w[2]), 1.0, 1.0, float(w[6]), float(w[7])]

    key = tuple(post) + (float(w[0]), float(w[4]))
    if key not in _CACHE:
        _CACHE[key] = _build(post)
    nc = _CACHE[key]

    opw_eff = np.stack(Wm).astype(np.float32)          # (NK, C, C)
    opb_eff = np.stack(bm).astype(np.float32).T        # (C, NK)

    # rw: (V, C) -> [p, vt, c];  opw: (NK, C, C) -> [p, k, c]
    rwH = np.ascontiguousarray(
        read_w.reshape(NV, 128, C).transpose(1, 0, 2)).reshape(128, NV * C)
    opwH = np.ascontiguousarray(
        opw_eff.transpose(1, 0, 2)).reshape(C, NK * C)

    S = 1024.0
    shared = {
        "rw": rwH.astype(bf16),
        "wwT": (wwT * S).astype(bf16),
        "opw": opwH.astype(bf16),
        "opb": np.ascontiguousarray(opb_eff),
    }
    # x[b] (T, V) -> [p, tcn, blk, j, tt] with v = (blk*VB + j)*128 + p
    x16 = x.astype(ml_dtypes.float8_e4m3)
    in_maps = []
    for b in range(B):
        xb = x16[b].reshape(NTC, 512, NBLK, VB, 128).transpose(4, 0, 2, 3, 1)
        m = dict(shared)
        m["xh"] = np.ascontiguousarray(xb).reshape(128, NTC * NBLK * VB * 512)
        in_maps.append(m)

    res = run_bass_kernel_spmd(nc, in_maps, core_ids=list(range(NCORES)))
    LAST_RESULT = res
    out = np.empty((B, T, V), np.float32)
    for b in range(B):
        d = np.asarray(res.results[b]["out"], np.float32) / S
        negsum = np.asarray(res.results[b]["asum"], np.float32)  # (C, NTC)
        mean = -negsum / 512.0
        base = mean.T @ wwT                                      # (NTC, V)
        out[b] = (d.reshape(NTC, 512, V) + base[:, None, :]).reshape(T, V)
    return out
